# revision 17
# baseline (speedup 1.0000x reference)
"""Classical self-attention (single head) on 8 Trainium2 NeuronCores.

Reference computation (fp32):
    Q = X @ R; K = X @ E
    out = softmax(Q K^T / sqrt(D)) @ X
with X [4, 2048, 512], R/E [512, 512].

Sharding: 8 cores = 4 batches x 2 sequence halves (data parallel, no
collectives). Each core receives its batch's X twice (natural layout for
the attention-value matmul, transposed layout for the projections), with
the sequence rows rotated so the core's own query half occupies rows
0..1023 — this keeps the SPMD program identical across cores.

Precision strategy: the softmax here is near-one-hot (logit std ~600)
with occasional near-tie rows, so logits must be fp32-grade. Plain fp32
matmuls run at 4 cycles/row on the PE; instead the projection and logits
matmuls use an fp16 hi/lo decomposition (a = ah + al with ah = fp16(a)):
a.b = ah.bh + ah.bl + al.bh to ~2^-22 relative accuracy, i.e. 3 passes at
1 cycle/row = 0.75x the fp32 cost. The attention-value matmul
(scores @ X) uses single-pass float32r (~13-bit mantissa): scores are
<= 1 and X is O(1), so the added output error is ~1e-4 absolute, well
inside the fp32 softmax envelope of this problem.

Per-core program:
  1. K^T[e, s] and Q^T[e, q] via 3-pass fp16 matmuls (contract d on
     partitions), with hi/lo fp16 splits of X^T, R, E prepared on host.
     K^T/Q^T are themselves stored as fp16 hi/lo pairs (ACT rounds the
     PSUM to hi, DVE subtracts to form lo).
  2. Per 128-row query block: logits via 3-pass fp16 matmuls (contract
     e), row max via DVE, exp((L - max) * scale) on ACT with fused
     row-sum accumulation, PE-transpose of the fp32 probability tile
     (rounded to float32r on the PSUM->SBUF copy), float32r AV matmul
     (contract k), and a final 1/rowsum scaling fused into the
     PSUM->SBUF copy. Query blocks are software-pipelined so the PE
     never waits on softmax.
"""

import numpy as np

import concourse.bass as bass
import concourse.mybir as mybir
from concourse import bacc
from concourse.tile import TileContext
from concourse.masks import make_identity

B = 4
S = 2048
D = 512
P = 128
HALF = S // 2  # query rows per core
N_CORES = 8
SCALE = float(1.0 / np.sqrt(np.float32(D)))
FP32 = mybir.dt.float32
FP32R = mybir.dt.float32r
FP16 = mybir.dt.float16

DC = D // P  # 4 contraction chunks of 128 over d or e
KT_TILES = S // 512  # 4 free-dim tiles of 512 over keys
QT_TILES = HALF // 512  # 2 free-dim tiles of 512 over this core's queries
QB = HALF // P  # 8 query blocks of 128 rows
KC = S // P  # 16 key chunks of 128 for the AV contraction


def _build_nc(reps: int = 1, body: str = "all") -> bass.Bass:
    """Build the per-core program. reps>1 repeats the compute body inside the
    NEFF (same outputs overwritten) — used only to measure HW time by slope.
    body selects which part repeats: "all", "proj" (stage B only), or
    "main" (query-block loop only)."""
    nc = bacc.Bacc()
    x = nc.declare_dram_parameter("x", [S, D], FP32, isOutput=False)
    xt_h = nc.declare_dram_parameter("xt_h", [D, S], FP16, isOutput=False)
    xt_l = nc.declare_dram_parameter("xt_l", [D, S], FP16, isOutput=False)
    r_h = nc.declare_dram_parameter("r_h", [D, D], FP16, isOutput=False)
    r_l = nc.declare_dram_parameter("r_l", [D, D], FP16, isOutput=False)
    e_h = nc.declare_dram_parameter("e_h", [D, D], FP16, isOutput=False)
    e_l = nc.declare_dram_parameter("e_l", [D, D], FP16, isOutput=False)
    o = nc.declare_dram_parameter("o", [HALF, D], FP32, isOutput=True)

    with TileContext(nc) as tc:
        with (
            tc.tile_pool(name="resident", bufs=1) as rp,
            tc.tile_pool(name="work", bufs=2) as wp,
            tc.tile_pool(name="psum", bufs=2, space="PSUM") as pp,
        ):
            ident = rp.tile([P, P], FP16, tag="ident")
            make_identity(nc, ident)

            # ---- resident loads ----
            xth_sb, xtl_sb = [], []
            eh_sb, el_sb, rh_sb, rl_sb = [], [], [], []
            for i in range(DC):
                rows = slice(P * i, P * (i + 1))
                t = rp.tile([P, S], FP16, tag=f"xth{i}", name=f"xth{i}")
                nc.sync.dma_start(out=t, in_=xt_h[rows, :])
                xth_sb.append(t)
                t = rp.tile([P, S], FP16, tag=f"xtl{i}", name=f"xtl{i}")
                nc.sync.dma_start(out=t, in_=xt_l[rows, :])
                xtl_sb.append(t)
                t = rp.tile([P, D], FP16, tag=f"eh{i}", name=f"eh{i}")
                nc.sync.dma_start(out=t, in_=e_h[rows, :])
                eh_sb.append(t)
                t = rp.tile([P, D], FP16, tag=f"el{i}", name=f"el{i}")
                nc.sync.dma_start(out=t, in_=e_l[rows, :])
                el_sb.append(t)
                t = rp.tile([P, D], FP16, tag=f"rh{i}", name=f"rh{i}")
                nc.sync.dma_start(out=t, in_=r_h[rows, :])
                rh_sb.append(t)
                t = rp.tile([P, D], FP16, tag=f"rl{i}", name=f"rl{i}")
                nc.sync.dma_start(out=t, in_=r_l[rows, :])
                rl_sb.append(t)
            # X natural, rounded to float32r on-device for the AV matmul
            xr_sb = []
            for j in range(KC):
                stage = wp.tile([P, D], FP32, tag="xstage", name="xstage", bufs=3)
                nc.sync.dma_start(out=stage, in_=x[P * j : P * (j + 1), :])
                t = rp.tile([P, D], FP32R, tag=f"xr{j}", name=f"xr{j}")
                nc.vector.tensor_copy(out=t, in_=stage)
                xr_sb.append(t)

            # ---- stage B: K^T = E^T X^T, Q^T = R^T X^T (contract d) ----
            # outputs stored as fp16 hi/lo pairs for the 3-pass logits matmul
            kth_sb = [rp.tile([P, S], FP16, tag=f"kth{i}", name=f"kth{i}") for i in range(DC)]
            ktl_sb = [rp.tile([P, S], FP16, tag=f"ktl{i}", name=f"ktl{i}") for i in range(DC)]
            qth_sb = [rp.tile([P, HALF], FP16, tag=f"qth{i}", name=f"qth{i}") for i in range(DC)]
            qtl_sb = [rp.tile([P, HALF], FP16, tag=f"qtl{i}", name=f"qtl{i}") for i in range(DC)]

            def emit_proj(eb, nt, wh_sb, wl_sb, outh, outl):
                ecols = slice(P * eb, P * (eb + 1))
                ncols = slice(512 * nt, 512 * (nt + 1))
                bps = pp.tile([P, 512], FP32, tag="bps", name="bps", bufs=2)
                n_mm = 3 * DC
                i_mm = 0
                for dc in range(DC):
                    for wt, xt_t in (
                        (wh_sb[dc], xth_sb[dc]),
                        (wh_sb[dc], xtl_sb[dc]),
                        (wl_sb[dc], xth_sb[dc]),
                    ):
                        nc.tensor.matmul(
                            bps,
                            wt[:, ecols],
                            xt_t[:, ncols],
                            start=(i_mm == 0),
                            stop=(i_mm == n_mm - 1),
                        )
                        i_mm += 1
                nc.scalar.copy(out=outh[:, ncols], in_=bps)
                nc.vector.tensor_sub(outl[:, ncols], bps, outh[:, ncols])

            proj_reps = reps if body in ("all", "proj") else 1
            main_reps = reps if body in ("all", "main") else (0 if body == "proj" else 1)
            for _rep in range(proj_reps):
                for eb in range(DC):
                    for nt in range(KT_TILES):
                        emit_proj(eb, nt, eh_sb, el_sb, kth_sb[eb], ktl_sb[eb])
                    for nt in range(QT_TILES):
                        emit_proj(eb, nt, rh_sb, rl_sb, qth_sb[eb], qtl_sb[eb])
            for _rep in range(main_reps):

                # ---- main loop over query blocks, software-pipelined ----
                def emit_logits(qb):
                    qcols = slice(P * qb, P * (qb + 1))
                    l_sb = wp.tile([P, S], FP32, tag="l_sb", name="l_sb", bufs=2)
                    m_part = wp.tile([P, KT_TILES], FP32, tag="m_part", name="m_part", bufs=2)
                    for kt in range(KT_TILES):
                        ncols = slice(512 * kt, 512 * (kt + 1))
                        lps = pp.tile([P, 512], FP32, tag="lps", name="lps", bufs=2)
                        n_mm = 3 * DC
                        i_mm = 0
                        for ec in range(DC):
                            for qt_t, kt_t in (
                                (qth_sb[ec], kth_sb[ec]),
                                (qth_sb[ec], ktl_sb[ec]),
                                (qtl_sb[ec], kth_sb[ec]),
                            ):
                                nc.tensor.matmul(
                                    lps,
                                    qt_t[:, qcols],
                                    kt_t[:, ncols],
                                    start=(i_mm == 0),
                                    stop=(i_mm == n_mm - 1),
                                )
                                i_mm += 1
                        nc.scalar.copy(out=l_sb[:, ncols], in_=lps)
                        nc.vector.reduce_max(
                            out=m_part[:, kt : kt + 1], in_=lps, axis=mybir.AxisListType.X
                        )
                    m_row = wp.tile([P, 1], FP32, tag="m_row", name="m_row", bufs=2)
                    nc.vector.reduce_max(out=m_row, in_=m_part, axis=mybir.AxisListType.X)
                    negb = wp.tile([P, 1], FP32, tag="negb", name="negb", bufs=2)
                    nc.scalar.mul(negb, m_row, -SCALE)
                    p_sb = wp.tile([P, S], FP16, tag="p_sb", name="p_sb", bufs=2)
                    s_part = wp.tile([P, KT_TILES], FP32, tag="s_part", name="s_part", bufs=2)
                    for kt in range(KT_TILES):
                        ncols = slice(512 * kt, 512 * (kt + 1))
                        nc.scalar.activation(
                            out=p_sb[:, ncols],
                            in_=l_sb[:, ncols],
                            func=mybir.ActivationFunctionType.Exp,
                            bias=negb,
                            scale=SCALE,
                            accum_out=s_part[:, kt : kt + 1],
                        )
                    s_row = wp.tile([P, 1], FP32, tag="s_row", name="s_row", bufs=2)
                    nc.vector.reduce_sum(out=s_row, in_=s_part, axis=mybir.AxisListType.X)
                    rinv = wp.tile([P, 1], FP32, tag="rinv", name="rinv", bufs=2)
                    nc.vector.reciprocal(rinv, s_row)
                    return p_sb, rinv

                def emit_attend(qb, p_sb, rinv):
                    pt_sb = wp.tile([P, KC, P], FP32R, tag="pt_sb", name="pt_sb", bufs=2)
                    for kc in range(KC):
                        tps = pp.tile([P, P], FP16, tag="tps", name="tps", bufs=2)
                        nc.tensor.transpose(tps, p_sb[:, P * kc : P * (kc + 1)], ident)
                        nc.vector.tensor_copy(out=pt_sb[:, kc, :], in_=tps)
                    ops = pp.tile([P, D], FP32, tag="ops", name="ops", bufs=2)
                    for kc in range(KC):
                        nc.tensor.matmul(
                            ops,
                            pt_sb[:, kc, :],
                            xr_sb[kc],
                            start=(kc == 0),
                            stop=(kc == KC - 1),
                        )
                    o_sb = wp.tile([P, D], FP32, tag="o_sb", name="o_sb", bufs=2)
                    nc.scalar.mul(o_sb, ops, rinv)
                    nc.sync.dma_start(out=o[P * qb : P * (qb + 1), :], in_=o_sb)

                prev = None
                for qb in range(QB):
                    cur = emit_logits(qb)
                    if prev is not None:
                        emit_attend(qb - 1, *prev)
                    prev = cur
                emit_attend(QB - 1, *prev)

    nc.finalize()
    return nc


_CACHE = {}


def _get_runner():
    """Build the Bass program once and wrap it in a cached jitted SPMD call."""
    if "runner" in _CACHE:
        return _CACHE["runner"]
    _CACHE["runner"] = _build_jit(_build_nc())
    return _CACHE["runner"]


def _build_jit(nc):
    """Wrap a finalized Bass module in a jitted 8-core SPMD callable.

    Mirrors concourse.bass2jax.run_bass_via_pjrt, but keeps one jitted
    callable alive so repeated calls don't recompile.
    """
    import jax
    from jax.sharding import Mesh, PartitionSpec
    from jax.experimental.shard_map import shard_map
    from concourse import bass2jax

    bass2jax.install_neuronx_cc_hook()

    partition_name = nc.partition_id_tensor.name if nc.partition_id_tensor else None
    in_names = []
    out_names = []
    out_avals = []
    for alloc in nc.m.functions[0].allocations:
        if not isinstance(alloc, mybir.MemoryLocationSet):
            continue
        name = alloc.memorylocations[0].name
        if alloc.kind == "ExternalInput":
            if name != partition_name:
                in_names.append(name)
        elif alloc.kind == "ExternalOutput":
            out_names.append(name)
            out_avals.append(
                jax.core.ShapedArray(tuple(alloc.tensor_shape), mybir.dt.np(alloc.dtype))
            )
    n_params = len(in_names)
    all_names = in_names + out_names
    if partition_name is not None:
        all_names = all_names + [partition_name]

    def _body(*args):
        operands = list(args)
        if partition_name is not None:
            operands.append(bass2jax.partition_id_tensor())
        outs = bass2jax._bass_exec_p.bind(
            *operands,
            out_avals=tuple(out_avals),
            in_names=tuple(all_names),
            out_names=tuple(out_names),
            lowering_input_output_aliases=(),
            sim_require_finite=True,
            sim_require_nnan=True,
            nc=nc,
        )
        return tuple(outs)

    devices = jax.devices()[:N_CORES]
    assert len(devices) == N_CORES, f"need {N_CORES} cores, have {len(jax.devices())}"
    mesh = Mesh(np.asarray(devices), ("core",))
    n_outs = len(out_names)
    sharded = jax.jit(
        shard_map(
            _body,
            mesh=mesh,
            in_specs=(PartitionSpec("core"),) * (n_params + n_outs),
            out_specs=(PartitionSpec("core"),) * n_outs,
            check_rep=False,
        ),
        keep_unused=True,
    )

    zero_outs = [
        np.zeros((N_CORES * a.shape[0], *a.shape[1:]), a.dtype) for a in out_avals
    ]

    def run(in_maps):
        concat_in = [
            np.concatenate([np.asarray(m[name]) for m in in_maps], axis=0)
            for name in in_names
        ]
        out_arrs = sharded(*concat_in, *zero_outs)
        return [
            {
                name: np.asarray(out_arrs[i]).reshape(
                    N_CORES, *out_avals[i].shape
                )[c]
                for i, name in enumerate(out_names)
            }
            for c in range(N_CORES)
        ]

    return (run, sharded, in_names, out_names, out_avals)


def _split_f16(a):
    hi = a.astype(np.float16)
    lo = (a - hi.astype(np.float32)).astype(np.float16)
    return hi, lo


def make_in_maps(inputs, rotation, entangle):
    inputs = np.asarray(inputs, dtype=np.float32)
    rotation = np.ascontiguousarray(rotation, dtype=np.float32)
    entangle = np.ascontiguousarray(entangle, dtype=np.float32)
    r_h, r_l = _split_f16(rotation)
    e_h, e_l = _split_f16(entangle)
    in_maps = []
    for c in range(N_CORES):
        b, h = divmod(c, 2)
        xb = inputs[b]
        if h == 0:
            perm = xb
        else:
            perm = np.concatenate([xb[HALF:], xb[:HALF]], axis=0)
        perm = np.ascontiguousarray(perm)
        xt = np.ascontiguousarray(perm.T)
        xt_h, xt_l = _split_f16(xt)
        in_maps.append(
            {
                "x": perm,
                "xt_h": xt_h,
                "xt_l": xt_l,
                "r_h": r_h,
                "r_l": r_l,
                "e_h": e_h,
                "e_l": e_l,
            }
        )
    return in_maps


def kernel(inputs, rotation, entangle):
    run = _get_runner()[0]
    in_maps = make_in_maps(inputs, rotation, entangle)
    results = run(in_maps)
    out = np.empty((B, S, D), dtype=np.float32)
    for c in range(N_CORES):
        b, h = divmod(c, 2)
        out[b, HALF * h : HALF * (h + 1)] = results[c]["o"]
    return out


# revision 19
# speedup vs baseline: 1.2015x; 1.2015x over previous
"""Classical self-attention (single head) on 8 Trainium2 NeuronCores.

Reference computation (fp32):
    Q = X @ R; K = X @ E
    out = softmax(Q K^T / sqrt(D)) @ X
with X [4, 2048, 512], R/E [512, 512].

Sharding: 8 cores = 4 batches x 2 sequence halves (data parallel, no
collectives). Each core receives its batch's X twice (natural layout for
the attention-value matmul, transposed layout for the projections), with
the sequence rows rotated so the core's own query half occupies rows
0..1023 — this keeps the SPMD program identical across cores.

Precision strategy: the softmax here is near-one-hot (logit std ~600)
with occasional near-tie rows, so logits must be fp32-grade. Plain fp32
matmuls run at 4 cycles/row on the PE; instead the projection and logits
matmuls use an fp16 hi/lo decomposition (a = ah + al with ah = fp16(a)):
a.b = ah.bh + ah.bl + al.bh to ~2^-22 relative accuracy, i.e. 3 passes at
1 cycle/row = 0.75x the fp32 cost. The attention-value matmul
(scores @ X) uses single-pass float32r (~13-bit mantissa): scores are
<= 1 and X is O(1), so the added output error is ~1e-4 absolute, well
inside the fp32 softmax envelope of this problem.

Per-core program:
  1. K^T[e, s] and Q^T[e, q] via 3-pass fp16 matmuls (contract d on
     partitions), with hi/lo fp16 splits of X^T, R, E prepared on host.
     K^T/Q^T are themselves stored as fp16 hi/lo pairs (ACT rounds the
     PSUM to hi, DVE subtracts to form lo).
  2. Per 128-row query block: logits via 3-pass fp16 matmuls (contract
     e), row max via DVE, exp((L - max) * scale) on ACT with fused
     row-sum accumulation, PE-transpose of the fp32 probability tile
     (rounded to float32r on the PSUM->SBUF copy), float32r AV matmul
     (contract k), and a final 1/rowsum scaling fused into the
     PSUM->SBUF copy. Query blocks are software-pipelined so the PE
     never waits on softmax.
"""

import numpy as np

import concourse.bass as bass
import concourse.mybir as mybir
from concourse import bacc
from concourse.tile import TileContext
from concourse.masks import make_identity

B = 4
S = 2048
D = 512
P = 128
HALF = S // 2  # query rows per core
N_CORES = 8
SCALE = float(1.0 / np.sqrt(np.float32(D)))
FP32 = mybir.dt.float32
FP32R = mybir.dt.float32r
FP16 = mybir.dt.float16

DC = D // P  # 4 contraction chunks of 128 over d or e
KT_TILES = S // 512  # 4 free-dim tiles of 512 over keys
QT_TILES = HALF // 512  # 2 free-dim tiles of 512 over this core's queries
QB = HALF // P  # 8 query blocks of 128 rows
KC = S // P  # 16 key chunks of 128 for the AV contraction


def _build_nc(reps: int = 1, body: str = "all") -> bass.Bass:
    """Build the per-core program. reps>1 repeats the compute body inside the
    NEFF (same outputs overwritten) — used only to measure HW time by slope.
    body selects which part repeats: "all", "proj" (stage B only), or
    "main" (query-block loop only)."""
    nc = bacc.Bacc()
    x = nc.declare_dram_parameter("x", [S, D], FP32, isOutput=False)
    xt_h = nc.declare_dram_parameter("xt_h", [D, S], FP16, isOutput=False)
    xt_l = nc.declare_dram_parameter("xt_l", [D, S], FP16, isOutput=False)
    r_h = nc.declare_dram_parameter("r_h", [D, D], FP16, isOutput=False)
    r_l = nc.declare_dram_parameter("r_l", [D, D], FP16, isOutput=False)
    e_h = nc.declare_dram_parameter("e_h", [D, D], FP16, isOutput=False)
    e_l = nc.declare_dram_parameter("e_l", [D, D], FP16, isOutput=False)
    o = nc.declare_dram_parameter("o", [HALF, D], FP32, isOutput=True)

    with TileContext(nc) as tc:
        with (
            tc.tile_pool(name="resident", bufs=1) as rp,
            tc.tile_pool(name="work", bufs=2) as wp,
            tc.tile_pool(name="psum", bufs=2, space="PSUM") as pp,
        ):
            ident = rp.tile([P, P], FP16, tag="ident")
            make_identity(nc, ident)

            # ---- resident loads ----
            xth_sb, xtl_sb = [], []
            eh_sb, el_sb, rh_sb, rl_sb = [], [], [], []
            for i in range(DC):
                rows = slice(P * i, P * (i + 1))
                t = rp.tile([P, S], FP16, tag=f"xth{i}", name=f"xth{i}")
                nc.sync.dma_start(out=t, in_=xt_h[rows, :])
                xth_sb.append(t)
                t = rp.tile([P, S], FP16, tag=f"xtl{i}", name=f"xtl{i}")
                nc.sync.dma_start(out=t, in_=xt_l[rows, :])
                xtl_sb.append(t)
                t = rp.tile([P, D], FP16, tag=f"eh{i}", name=f"eh{i}")
                nc.sync.dma_start(out=t, in_=e_h[rows, :])
                eh_sb.append(t)
                t = rp.tile([P, D], FP16, tag=f"el{i}", name=f"el{i}")
                nc.sync.dma_start(out=t, in_=e_l[rows, :])
                el_sb.append(t)
                t = rp.tile([P, D], FP16, tag=f"rh{i}", name=f"rh{i}")
                nc.sync.dma_start(out=t, in_=r_h[rows, :])
                rh_sb.append(t)
                t = rp.tile([P, D], FP16, tag=f"rl{i}", name=f"rl{i}")
                nc.sync.dma_start(out=t, in_=r_l[rows, :])
                rl_sb.append(t)
            # X natural, rounded to float32r on-device for the AV matmul
            xr_sb = []
            for j in range(KC):
                stage = wp.tile([P, D], FP32, tag="xstage", name="xstage", bufs=3)
                nc.sync.dma_start(out=stage, in_=x[P * j : P * (j + 1), :])
                t = rp.tile([P, D], FP32R, tag=f"xr{j}", name=f"xr{j}")
                nc.vector.tensor_copy(out=t, in_=stage)
                xr_sb.append(t)

            # ---- stage B: K^T = E^T X^T, Q^T = R^T X^T (contract d) ----
            # outputs stored as fp16 hi/lo pairs for the 3-pass logits matmul
            kth_sb = [rp.tile([P, S], FP16, tag=f"kth{i}", name=f"kth{i}") for i in range(DC)]
            ktl_sb = [rp.tile([P, S], FP16, tag=f"ktl{i}", name=f"ktl{i}") for i in range(DC)]
            qth_sb = [rp.tile([P, HALF], FP16, tag=f"qth{i}", name=f"qth{i}") for i in range(DC)]
            qtl_sb = [rp.tile([P, HALF], FP16, tag=f"qtl{i}", name=f"qtl{i}") for i in range(DC)]

            def emit_proj(eb, nt, wh_sb, wl_sb, outh, outl):
                ecols = slice(P * eb, P * (eb + 1))
                ncols = slice(512 * nt, 512 * (nt + 1))
                bps = pp.tile([P, 512], FP32, tag="bps", name="bps", bufs=2)
                n_mm = 3 * DC
                i_mm = 0
                for dc in range(DC):
                    for wt, xt_t in (
                        (wh_sb[dc], xth_sb[dc]),
                        (wh_sb[dc], xtl_sb[dc]),
                        (wl_sb[dc], xth_sb[dc]),
                    ):
                        nc.tensor.matmul(
                            bps,
                            wt[:, ecols],
                            xt_t[:, ncols],
                            start=(i_mm == 0),
                            stop=(i_mm == n_mm - 1),
                        )
                        i_mm += 1
                nc.scalar.copy(out=outh[:, ncols], in_=bps)
                nc.vector.tensor_sub(outl[:, ncols], bps, outh[:, ncols])

            proj_reps = reps if body in ("all", "proj") else 1
            main_reps = reps if body in ("all", "main") else (0 if body == "proj" else 1)
            for _rep in range(proj_reps):
                for eb in range(DC):
                    for nt in range(KT_TILES):
                        emit_proj(eb, nt, eh_sb, el_sb, kth_sb[eb], ktl_sb[eb])
                    for nt in range(QT_TILES):
                        emit_proj(eb, nt, rh_sb, rl_sb, qth_sb[eb], qtl_sb[eb])
            for _rep in range(main_reps):

                # ---- main loop over query blocks, software-pipelined ----
                def emit_logits(qb):
                    qcols = slice(P * qb, P * (qb + 1))
                    l_sb = wp.tile([P, S], FP32, tag="l_sb", name="l_sb", bufs=2)
                    m_part = wp.tile([P, KT_TILES], FP32, tag="m_part", name="m_part", bufs=2)
                    for kt in range(KT_TILES):
                        ncols = slice(512 * kt, 512 * (kt + 1))
                        lps = pp.tile([P, 512], FP32, tag="lps", name="lps", bufs=2)
                        n_mm = 3 * DC
                        i_mm = 0
                        for ec in range(DC):
                            for qt_t, kt_t in (
                                (qth_sb[ec], kth_sb[ec]),
                                (qth_sb[ec], ktl_sb[ec]),
                                (qtl_sb[ec], kth_sb[ec]),
                            ):
                                nc.tensor.matmul(
                                    lps,
                                    qt_t[:, qcols],
                                    kt_t[:, ncols],
                                    start=(i_mm == 0),
                                    stop=(i_mm == n_mm - 1),
                                )
                                i_mm += 1
                        nc.scalar.copy(out=l_sb[:, ncols], in_=lps)
                        nc.vector.reduce_max(
                            out=m_part[:, kt : kt + 1], in_=lps, axis=mybir.AxisListType.X
                        )
                    m_row = wp.tile([P, 1], FP32, tag="m_row", name="m_row", bufs=2)
                    nc.vector.reduce_max(out=m_row, in_=m_part, axis=mybir.AxisListType.X)
                    negb = wp.tile([P, 1], FP32, tag="negb", name="negb", bufs=2)
                    nc.scalar.mul(negb, m_row, -SCALE)
                    p_sb = wp.tile([P, S], FP16, tag="p_sb", name="p_sb", bufs=2)
                    s_part = wp.tile([P, KT_TILES], FP32, tag="s_part", name="s_part", bufs=2)
                    for kt in range(KT_TILES):
                        ncols = slice(512 * kt, 512 * (kt + 1))
                        nc.scalar.activation(
                            out=p_sb[:, ncols],
                            in_=l_sb[:, ncols],
                            func=mybir.ActivationFunctionType.Exp,
                            bias=negb,
                            scale=SCALE,
                            accum_out=s_part[:, kt : kt + 1],
                        )
                    s_row = wp.tile([P, 1], FP32, tag="s_row", name="s_row", bufs=2)
                    nc.vector.reduce_sum(out=s_row, in_=s_part, axis=mybir.AxisListType.X)
                    rinv = wp.tile([P, 1], FP32, tag="rinv", name="rinv", bufs=2)
                    nc.vector.reciprocal(rinv, s_row)
                    return p_sb, rinv

                def emit_attend(qb, p_sb, rinv):
                    pt_sb = wp.tile([P, KC, P], FP32R, tag="pt_sb", name="pt_sb", bufs=2)
                    for kc in range(KC):
                        tps = pp.tile([P, P], FP16, tag="tps", name="tps", bufs=2)
                        nc.tensor.transpose(tps, p_sb[:, P * kc : P * (kc + 1)], ident)
                        nc.vector.tensor_copy(out=pt_sb[:, kc, :], in_=tps)
                    ops = pp.tile([P, D], FP32, tag="ops", name="ops", bufs=2)
                    for kc in range(KC):
                        nc.tensor.matmul(
                            ops,
                            pt_sb[:, kc, :],
                            xr_sb[kc],
                            start=(kc == 0),
                            stop=(kc == KC - 1),
                        )
                    o_sb = wp.tile([P, D], FP32, tag="o_sb", name="o_sb", bufs=2)
                    nc.scalar.mul(o_sb, ops, rinv)
                    nc.sync.dma_start(out=o[P * qb : P * (qb + 1), :], in_=o_sb)

                prev = None
                for qb in range(QB):
                    cur = emit_logits(qb)
                    if prev is not None:
                        emit_attend(qb - 1, *prev)
                    prev = cur
                emit_attend(QB - 1, *prev)

    nc.finalize()
    return nc


_CACHE = {}


def _get_runner():
    """Build the Bass program once and wrap it in a cached jitted SPMD call."""
    if "runner" in _CACHE:
        return _CACHE["runner"]
    _CACHE["runner"] = _build_jit(_build_nc())
    return _CACHE["runner"]


def _build_jit(nc):
    """Wrap a finalized Bass module in a jitted 8-core SPMD callable.

    Mirrors concourse.bass2jax.run_bass_via_pjrt, but keeps one jitted
    callable alive so repeated calls don't recompile.
    """
    import jax
    from jax.sharding import Mesh, PartitionSpec
    from jax.experimental.shard_map import shard_map
    from concourse import bass2jax

    bass2jax.install_neuronx_cc_hook()

    partition_name = nc.partition_id_tensor.name if nc.partition_id_tensor else None
    in_names = []
    out_names = []
    out_avals = []
    for alloc in nc.m.functions[0].allocations:
        if not isinstance(alloc, mybir.MemoryLocationSet):
            continue
        name = alloc.memorylocations[0].name
        if alloc.kind == "ExternalInput":
            if name != partition_name:
                in_names.append(name)
        elif alloc.kind == "ExternalOutput":
            out_names.append(name)
            out_avals.append(
                jax.core.ShapedArray(tuple(alloc.tensor_shape), mybir.dt.np(alloc.dtype))
            )
    n_params = len(in_names)
    all_names = in_names + out_names
    if partition_name is not None:
        all_names = all_names + [partition_name]

    def _body(*args):
        operands = list(args)
        if partition_name is not None:
            operands.append(bass2jax.partition_id_tensor())
        outs = bass2jax._bass_exec_p.bind(
            *operands,
            out_avals=tuple(out_avals),
            in_names=tuple(all_names),
            out_names=tuple(out_names),
            lowering_input_output_aliases=(),
            sim_require_finite=True,
            sim_require_nnan=True,
            nc=nc,
        )
        return tuple(outs)

    devices = jax.devices()[:N_CORES]
    assert len(devices) == N_CORES, f"need {N_CORES} cores, have {len(jax.devices())}"
    mesh = Mesh(np.asarray(devices), ("core",))
    n_outs = len(out_names)
    sharded = jax.jit(
        shard_map(
            _body,
            mesh=mesh,
            in_specs=(PartitionSpec("core"),) * (n_params + n_outs),
            out_specs=(PartitionSpec("core"),) * n_outs,
            check_rep=False,
        ),
        keep_unused=True,
    )

    zero_outs = [
        np.zeros((N_CORES * a.shape[0], *a.shape[1:]), a.dtype) for a in out_avals
    ]

    def run(in_maps):
        concat_in = [
            np.concatenate([np.asarray(m[name]) for m in in_maps], axis=0)
            for name in in_names
        ]
        out_arrs = sharded(*concat_in, *zero_outs)
        return [
            {
                name: np.asarray(out_arrs[i]).reshape(
                    N_CORES, *out_avals[i].shape
                )[c]
                for i, name in enumerate(out_names)
            }
            for c in range(N_CORES)
        ]

    return (run, sharded, in_names, out_names, out_avals)


def _split_f16(a):
    hi = a.astype(np.float16)
    lo = (a - hi.astype(np.float32)).astype(np.float16)
    return hi, lo


def make_in_maps(inputs, rotation, entangle):
    inputs = np.asarray(inputs, dtype=np.float32)
    rotation = np.ascontiguousarray(rotation, dtype=np.float32)
    entangle = np.ascontiguousarray(entangle, dtype=np.float32)
    r_h, r_l = _split_f16(rotation)
    e_h, e_l = _split_f16(entangle)
    in_maps = []
    for c in range(N_CORES):
        b, h = divmod(c, 2)
        xb = inputs[b]
        if h == 0:
            perm = xb
        else:
            perm = np.concatenate([xb[HALF:], xb[:HALF]], axis=0)
        perm = np.ascontiguousarray(perm)
        xt = np.ascontiguousarray(perm.T)
        xt_h, xt_l = _split_f16(xt)
        in_maps.append(
            {
                "x": perm,
                "xt_h": xt_h,
                "xt_l": xt_l,
                "r_h": r_h,
                "r_l": r_l,
                "e_h": e_h,
                "e_l": e_l,
            }
        )
    return in_maps


def kernel(inputs, rotation, entangle):
    run = _get_runner()[0]
    in_maps = make_in_maps(inputs, rotation, entangle)
    results = run(in_maps)
    out = np.empty((B, S, D), dtype=np.float32)
    for c in range(N_CORES):
        b, h = divmod(c, 2)
        out[b, HALF * h : HALF * (h + 1)] = results[c]["o"]
    return out
